# revision 37
# baseline (speedup 1.0000x reference)
# Bidirectional cross-attention Trainium2 kernel (Bass/Tile), 8-core head-parallel.
#
# Sharding: 16 heads / 8 cores = 2 heads per core (tensor parallel on h); each
# core computes its heads' projections, similarity, both softmax directions and
# its row-parallel partial of the final projections; host sums partials + bias.
#
# Design:
#  - everything 2-byte fp16 (fp8 fails the 2e-2 gate: elementwise quant noise
#    is preserved through random-sign dot products, measured 5.5e-2)
#  - exp computed ONCE per head (E stored fp16), [128,1024] psum tiles
#  - E^T via DMA xbar transposes emitted as two back-to-back bursts per
#    phase (mid-phase + phase end): bursts keep the fast xbar regime while
#    letting the next phase's g_half consumer start half a phase earlier
#    (measured ~-30us vs one burst per phase)
#  - softmax sums ride along as a ones-column in the V operands
#  - normalization: DVE reciprocal (f32r) + K=1 PE ones-broadcast + DVE mul
#  - half-major phase order (h0,j0)(h1,j0)(h0,j1)(h1,j1): each j-half's ctx
#    rows finish two phases in, so ctx-final projections stream during the
#    phases instead of tailing
#  - G accumulated per j-half into a persistent f32 SBUF accumulator (PSUM
#    chunks + DVE add): only the last head's last half + finals remain after
#    the phase loop
#  - x on the SP HWDGE queue / context on the ACT queue; qk+cqk projections
#    run as the first pass so the first sim phase starts as early as possible
#  - software-pipelined emission: H/G/norm/final work is sprinkled between
#    sim+exp iterations; tail drains FIFO (emission order = dependency order)
#  - phase-resident PSUM->SBUF copies (hT, ctx-final) run on DVE, not ACT:
#    an ACT copy inside the exp stream stalls the pacing engine (measured
#    ~-10us each for hT and ctx-final)

import os
import sys

for _p in ("/opt/trn_rl_repo", "/root/.axon_site/_ro/trn_rl_repo"):
    if os.path.isdir(_p) and _p not in sys.path:
        sys.path.insert(0, _p)

import numpy as np

SEQ_MODE = os.environ.get("KSEQ", "0") == "1"
SPRINKLE_N = int(os.environ.get("KSPR", "8"))
TSPLIT = int(os.environ.get("KTSPLIT", "1"))
WARM = os.environ.get("KWARM", "0") == "1"
HT_DVE = os.environ.get("KHTDVE", "1") == "1"
CF_DVE = os.environ.get("KCFDVE", "1") == "1"
ODMA_GP = os.environ.get("KODMA", "0") == "1"

HEADS = 16
DIM_HEAD = 64
DIM = 1024
SEQ = 2048
N_CORES = 8
HPC = HEADS // N_CORES          # heads per core = 2
FPC = HPC * DIM_HEAD            # feature cols per core = 128
SCALE = DIM_HEAD ** -0.5


def _ts(i, size):
    return slice(i * size, (i + 1) * size)


def build_bass(seq=SEQ, dim=DIM, fpc=FPC, hpc=HPC, num_devices=N_CORES, stage='full',
               reps=1, debug_dump=False):
    import concourse.bacc as bacc
    import concourse.tile as tile
    import concourse.mybir as mybir
    from contextlib import ExitStack

    f32 = mybir.dt.float32
    f16 = mybir.dt.float16
    Exp = mybir.ActivationFunctionType.Exp

    P = 128
    KT = dim // P              # contraction tiles over DIM (8)
    NT = seq // P              # 128-blocks along sequence (16)
    NCH = seq // 512           # 512-chunks along sequence (4)
    J2 = min(1024, seq)        # exp-tile width
    N2CH = seq // J2           # exp-tile chunks along sequence
    HPT = J2 // 512            # 512-halves per exp tile
    ITPC = NT // NCH           # i-tiles per 512-chunk (4)
    OCH = dim // 512           # 512-chunks of output dim (2)
    dh = DIM_HEAD
    vw = dh + 1

    nc = bacc.Bacc("TRN2", target_bir_lowering=False, debug=False,
                   num_devices=num_devices)

    xT = nc.dram_tensor("xT", (dim, seq), f16, kind="ExternalInput").ap()
    cT = nc.dram_tensor("cT", (dim, seq), f16, kind="ExternalInput").ap()
    wqk = nc.dram_tensor("wqk", (dim, fpc), f16, kind="ExternalInput").ap()
    wv = nc.dram_tensor("wv", (dim, fpc), f16, kind="ExternalInput").ap()
    wcqk = nc.dram_tensor("wcqk", (dim, fpc), f16, kind="ExternalInput").ap()
    wcv = nc.dram_tensor("wcv", (dim, fpc), f16, kind="ExternalInput").ap()
    wout = nc.dram_tensor("wout", (fpc, dim), f16, kind="ExternalInput").ap()
    wcout = nc.dram_tensor("wcout", (fpc, dim), f16, kind="ExternalInput").ap()
    out_p = nc.dram_tensor("out_p", (seq, dim), f16, kind="ExternalOutput").ap()
    ctx_p = nc.dram_tensor("ctx_p", (seq, dim), f16, kind="ExternalOutput").ap()
    dbg = {}
    if debug_dump:
        dbg["outmT_d"] = nc.dram_tensor("outmT_d", (P, seq), f16, kind="ExternalOutput").ap()
        dbg["ctxmT_d"] = nc.dram_tensor("ctxmT_d", (P, seq), f16, kind="ExternalOutput").ap()
        dbg["gsb0_d"] = nc.dram_tensor("gsb0_d", (vw, seq), f32, kind="ExternalOutput").ap()
        dbg["gsb1_d"] = nc.dram_tensor("gsb1_d", (vw, seq), f32, kind="ExternalOutput").ap()
        dbg["qkT_d"] = nc.dram_tensor("qkT_d", (P, seq), f16, kind="ExternalOutput").ap()
        dbg["v_d"] = nc.dram_tensor("v_d", (P, NT * hpc * vw), f16, kind="ExternalOutput").ap()

    with tile.TileContext(nc) as tc:
      for _rep in range(reps):
        with ExitStack() as ctx:
            sfx = f"_r{_rep}"
            persist = ctx.enter_context(tc.tile_pool(name="persist" + sfx, bufs=1))
            fin_pool = ctx.enter_context(tc.tile_pool(name="finpool" + sfx, bufs=4))

            qkT_sb = persist.tile([P, seq], f16, tag="qkT")
            cqkT_sb = persist.tile([P, seq], f16, tag="cqkT")
            v_sb = persist.tile([P, NT, hpc * vw], f16, tag="v")
            cv_sb = persist.tile([P, NT, hpc * vw], f16, tag="cv")
            wout_sb = persist.tile([P, dim], f16, tag="wout")
            wcout_sb = persist.tile([P, dim], f16, tag="wcout")
            outmT_sb = persist.tile([P, seq], f16, tag="outmT")
            ctxmT_sb = persist.tile([P, seq], f16, tag="ctxmT")
            f32r = mybir.dt.float32r
            ones_f = persist.tile([1, dh], f32, tag="onesf", name="ones_f" + sfx)
            nc.vector.memset(ones_f, 1.0)
            ones_r = persist.tile([1, dh], f32r, tag="ones", name="ones_r" + sfx)
            with nc.allow_low_precision(reason="ones constant, exact in f32r"):
                nc.vector.tensor_copy(ones_r, ones_f)

            nc.sync.dma_start(wout_sb, wout)
            nc.sync.dma_start(wcout_sb, wcout)

            # ---- load x/context + weights, compute projections, then release
            with tc.tile_pool(name="xcpool" + sfx, bufs=1) as xc_pool, \
                 tc.tile_pool(name="psproj" + sfx, bufs=8, space="PSUM") as ps_proj:
                w_sbs = {}
                for name, ap_, eng in (("wqk", wqk, nc.sync), ("wcqk", wcqk, nc.scalar),
                                       ("wv", wv, nc.sync), ("wcv", wcv, nc.scalar)):
                    t = xc_pool.tile([P, KT, fpc], f16, tag=name)
                    eng.dma_start(t, ap_.rearrange("(kt p) f -> p kt f", p=P))
                    w_sbs[name] = t
                xT_sb = xc_pool.tile([P, KT, seq], f16, tag="xT")
                cT_sb = xc_pool.tile([P, KT, seq], f16, tag="cT")
                xT_v = xT.rearrange("(kt p) i -> p kt i", p=P)
                cT_v = cT.rearrange("(kt p) i -> p kt i", p=P)
                # x chunks on the SP HWDGE queue, context on the ACT queue:
                # two dispatch streams keep both SDMA fan-outs busy
                for kt in range(KT):
                    nc.sync.dma_start(xT_sb[:, kt], xT_v[:, kt])
                    nc.scalar.dma_start(cT_sb[:, kt], cT_v[:, kt])

                from concourse.masks import make_identity
                ident = persist.tile([P, P], f16, tag="ident")
                make_identity(nc, ident)
                if WARM:
                    # keep the PE clocked up during the input load: ~40 dummy
                    # ident transposes (~2us) so the first projection matmuls
                    # start at the full 2.4GHz p-state instead of ramping
                    wps = ps_proj.tile([P, 512], f32, tag="pp",
                                       name="warm" + sfx)
                    wps16 = wps.bitcast(f16)
                    for wi in range(40):
                        nc.tensor.transpose(wps16[:, 0:P], ident, ident)
                for h in range(hpc):
                    nc.vector.memset(v_sb[:, :, h * vw + dh], 1.0)
                    nc.vector.memset(cv_sb[:, :, h * vw + dh], 1.0)
                vT_tmps = {}
                vT_tmps["wv"] = persist.tile([P, seq], f16, tag="vT_wv", name="vT_wv" + sfx)
                vT_tmps["wcv"] = persist.tile([P, seq], f16, tag="vT_wcv", name="vT_wcv" + sfx)
                # projections in two passes: qk+cqk first (they gate the
                # first sim phase), then v+cv; ktile-major so matmuls chase
                # the input DMAs; 8 psum accumulators live per pass
                proj_passes = (((xT_sb, "wqk", qkT_sb), (cT_sb, "wcqk", cqkT_sb)),
                               ((xT_sb, "wv", vT_tmps["wv"]),
                                (cT_sb, "wcv", vT_tmps["wcv"])))
                for cg, projs in enumerate(proj_passes if stage != 'load' else ()):
                    tiles = {}
                    for pi in range(2):
                        for cc in range(NCH):
                            tiles[(pi, cc)] = ps_proj.tile(
                                [P, 512], f32, tag="pp",
                                name=f"pp_{cg}_{pi}_{cc}" + sfx)
                    for kt in range(KT):
                        for pi, (src_sb, wname, dst) in enumerate(projs):
                            for cc in range(NCH):
                                nc.tensor.matmul(
                                    tiles[(pi, cc)], w_sbs[wname][:, kt],
                                    src_sb[:, kt, _ts(cc, 512)],
                                    start=(kt == 0), stop=(kt == KT - 1))
                    for pi, (src_sb, wname, dst) in enumerate(projs):
                        for cc in range(NCH):
                            nc.vector.tensor_copy(dst[:, _ts(cc, 512)],
                                                  tiles[(pi, cc)])
            ps_pool = ctx.enter_context(
                tc.tile_pool(name="pspool" + sfx, bufs=2, space="PSUM"))
            ps_acc = ctx.enter_context(
                tc.tile_pool(name="psacc" + sfx, bufs=2, space="PSUM"))

            # ---- per-head attention (software-pipelined emission) ----
            # Phases = (head, j-half). Each phase emits sim+exp+transpose for
            # 16 i-tiles; H/G accumulation and normalization work from earlier
            # phases is sprinkled between iterations so PE work rides under
            # the ACT-bound exp stream.
            from collections import deque

            e_pool = ctx.enter_context(tc.tile_pool(name="epool" + sfx, bufs=2))
            et_pool = ctx.enter_context(tc.tile_pool(name="etpool" + sfx, bufs=2))
            hg_pool = ctx.enter_context(tc.tile_pool(name="hgpool" + sfx, bufs=2))
            norm_pool = ctx.enter_context(tc.tile_pool(name="normpool" + sfx, bufs=2))
            JPH = NT // N2CH           # j-tiles per half (8)

            # per-head f32 G accumulators (value rows + sums row), built up
            # half-by-half so the out-side work streams instead of tailing
            G_sb = [persist.tile([vw, seq], f32, tag=f"gsb{h}",
                                 name=f"G_sb{h}" + sfx)
                    for h in range(hpc)]

            def vcv_transpose_work():
                for wname, dst in (("wv", v_sb), ("wcv", cv_sb)):
                    vT_tmp = vT_tmps[wname]
                    for ibg in range(NT // 4):
                        pst = ps_pool.tile([P, 1024], f32, tag="ps")
                        pst16 = pst.bitcast(f16)
                        for k in range(4):
                            nc.tensor.transpose(pst16[:, _ts(k, P)],
                                                vT_tmp[:, _ts(ibg * 4 + k, P)],
                                                ident)
                            yield
                        pstv = pst16[:, :4 * P].rearrange("p (k f) -> p k f", k=4)
                        for h in range(hpc):
                            nc.vector.tensor_copy(
                                dst[:, ibg * 4:(ibg + 1) * 4, h * vw:h * vw + dh],
                                pstv[:, :, h * dh:(h + 1) * dh])
                        yield

            def h_work(h, half, E_half):
                """Accumulate H^T chunks of this (head, j-half) + ctx norm."""
                hs = slice(h * dh, (h + 1) * dh)
                va = slice(h * vw, h * vw + vw)
                hT = hg_pool.tile([vw, J2], f16, tag="ht")
                psH = ps_acc.tile([vw, J2], f32, tag="acc")
                for jcc in range(HPT):
                    jsl_l = _ts(jcc, 512)
                    for it in range(NT):
                        nc.tensor.matmul(psH[:, jsl_l], v_sb[:, it, va],
                                         E_half[:, it, jsl_l],
                                         start=(it == 0), stop=(it == NT - 1))
                        yield
                    if HT_DVE:
                        nc.vector.tensor_copy(hT[:, jsl_l], psH[:, jsl_l])
                    else:
                        nc.scalar.copy(hT[:, jsl_l], psH[:, jsl_l])
                    rcs_r = norm_pool.tile([1, 512], f32r, tag="rc")
                    with nc.allow_low_precision(reason="softmax sums O(2e3); f32r rounding is ~1e-7 rel"):
                        nc.vector.reciprocal(rcs_r, hT[dh:dh + 1, jsl_l])
                    jsl_g = _ts(half * HPT + jcc, 512)
                    bc = ps_pool.tile([P, 1024], f32, tag="ps")
                    nc.tensor.matmul(bc[:dh, :512], ones_r, rcs_r,
                                     start=True, stop=True)
                    nc.vector.tensor_mul(ctxmT_sb[hs, jsl_g], hT[0:dh, jsl_l],
                                         bc[:dh, :512])
                    yield

            def g_half_work(h, half, eTh):
                """Accumulate this j-half's G contribution into G_sb[h]."""
                va = slice(h * vw, h * vw + vw)
                for ihalf in range(N2CH):
                    psG = ps_acc.tile([vw, J2], f32, tag="acc")
                    for icc in range(HPT):
                        isl_l = _ts(icc, 512)
                        isl_g = _ts(ihalf * HPT + icc, 512)
                        for jl in range(JPH):
                            nc.tensor.matmul(psG[:, isl_l], cv_sb[:, half * JPH + jl, va],
                                             eTh[:, jl, isl_g],
                                             start=(jl == 0), stop=(jl == JPH - 1))
                            yield
                    isl_gw = _ts(ihalf, J2)
                    if half == 0:
                        nc.vector.tensor_copy(G_sb[h][:, isl_gw], psG)
                    else:
                        nc.vector.tensor_add(G_sb[h][:, isl_gw], G_sb[h][:, isl_gw],
                                             psG)
                    yield

            def g_fin_work(h):
                """Normalize G_sb[h] into outmT (out-side norm)."""
                hs = slice(h * dh, (h + 1) * dh)
                for icc in range(NCH):
                    isl = _ts(icc, 512)
                    rrs_r = norm_pool.tile([1, 512], f32r, tag="rr")
                    with nc.allow_low_precision(reason="softmax sums O(2e3); f32r rounding is ~1e-7 rel"):
                        nc.vector.reciprocal(rrs_r, G_sb[h][dh:dh + 1, isl])
                    bc2 = ps_pool.tile([P, 1024], f32, tag="ps")
                    nc.tensor.matmul(bc2[:dh, :512], ones_r, rrs_r,
                                     start=True, stop=True)
                    nc.vector.tensor_mul(outmT_sb[hs, isl], G_sb[h][0:dh, isl],
                                         bc2[:dh, :512])
                    yield

            def final_work(mT, w_sb, odram, ibs, all_dve=False):
                for n, ib in enumerate(ibs):
                    pso = ps_pool.tile([P, 1024], f32, tag="ps")
                    for oc in range(OCH):
                        nc.tensor.matmul(pso[:, _ts(oc, 512)], mT[:, _ts(ib, P)],
                                         w_sb[:, _ts(oc, 512)],
                                         start=True, stop=True)
                        yield
                    osb = fin_pool.tile([P, dim], f16, tag="osb")
                    # phase-resident copies go on DVE (an ACT copy would stall
                    # the exp stream that paces the phases); tail copies
                    # alternate ACT/DVE so neither engine serializes the tail
                    if all_dve or n % 2 == 1:
                        nc.vector.tensor_copy(osb, pso[:, :dim])
                    else:
                        nc.scalar.copy(osb, pso[:, :dim])
                    if ODMA_GP:
                        # output DMAs via the idle SWDGE queue so they never
                        # queue behind the eT transpose bursts on SP-HWDGE
                        nc.gpsimd.dma_start(odram[:, ib, :], osb)
                    else:
                        nc.sync.dma_start(odram[:, ib, :], osb)
                    yield

            out_view = out_p.rearrange("(ib p) o -> p ib o", p=P)
            ctx_view = ctx_p.rearrange("(ib p) o -> p ib o", p=P)

            pending = deque()
            if stage != 'load':
                pending.append(vcv_transpose_work())

            def sprinkle(n):
                done = 0
                while pending and done < n:
                    try:
                        next(pending[0])
                        done += 1
                    except StopIteration:
                        pending.popleft()

            # half-major phase order: both heads of a j-half complete
            # back-to-back, so ctx-final columns stream out per half and the
            # per-half G accumulation keeps the out-side off the tail
            phases = [(h, half) for half in range(N2CH) for h in range(hpc)]
            if stage in ('load', 'proj'):
                phases = []
            do_hg = stage not in ('e0', 'e')
            do_fin = stage == 'full'
            for h, half in phases:
                hs = slice(h * dh, (h + 1) * dh)
                E_half = e_pool.tile([P, NT, J2], f16, tag="e")
                eTh = None
                if stage != 'e0':
                    eTh = et_pool.tile([P, JPH, seq], f16, tag="et")
                for it in range(NT):
                    ps = ps_pool.tile([P, 1024], f32, tag="ps")
                    for hlf in range(HPT):
                        js = _ts(half * HPT + hlf, 512)
                        nc.tensor.matmul(ps[:, _ts(hlf, 512)],
                                         qkT_sb[hs, _ts(it, P)],
                                         cqkT_sb[hs, js],
                                         start=True, stop=True)
                    nc.scalar.activation(E_half[:, it, :], ps[:, :J2],
                                         Exp, scale=SCALE)
                    if stage != 'e0' and TSPLIT == 1 and it == NT // 2 - 1:
                        for it2 in range(NT // 2):
                            nc.sync.dma_start_transpose(eTh[:, :, _ts(it2, P)],
                                                        E_half[:, it2, :])
                    if stage != 'e0' and TSPLIT == 2 and it % 4 == 3 and it < NT - 1:
                        for it2 in range(it - 3, it + 1):
                            nc.sync.dma_start_transpose(eTh[:, :, _ts(it2, P)],
                                                        E_half[:, it2, :])
                    sprinkle(SPRINKLE_N)
                if stage != 'e0':
                    # DMA xbar transposes in two back-to-back bursts (the
                    # fast regime) emitted mid-phase and at phase end, so the
                    # consumer (next phase's g_half) can start on the first
                    # i-half earlier
                    t0 = {0: 0, 1: NT // 2, 2: NT - 4}[TSPLIT]
                    for it in range(t0, NT):
                        nc.sync.dma_start_transpose(eTh[:, :, _ts(it, P)],
                                                    E_half[:, it, :])
                if do_hg:
                    pending.append(h_work(h, half, E_half))
                    pending.append(g_half_work(h, half, eTh))
                    if half == N2CH - 1:
                        pending.append(g_fin_work(h))
                if do_fin and h == hpc - 1:
                    # both heads' H for this j-half are now queued ahead in
                    # FIFO order; this half's ctx-final blocks follow them
                    jb0 = half * (NT // N2CH)
                    pending.append(final_work(ctxmT_sb, wcout_sb, ctx_view,
                                              range(jb0, jb0 + NT // N2CH),
                                              all_dve=CF_DVE and half < N2CH - 1))
                # sequential mode: drain phase work here (coarse-grained sync)
                if SEQ_MODE:
                    while pending:
                        sprinkle(1 << 30)

            if do_fin:
                pending.append(final_work(outmT_sb, wout_sb, out_view, range(NT)))
            else:
                while pending:
                    try:
                        next(pending[0])
                    except StopIteration:
                        pending.popleft()
                dummy = fin_pool.tile([P, dim], f16, tag="osb", name="dummy" + sfx)
                nc.vector.memset(outmT_sb, 0.0)
                nc.vector.memset(ctxmT_sb, 0.0)
                for h in range(hpc):
                    nc.vector.memset(G_sb[h], 0.0)
                nc.vector.memset(dummy, 0.0)
                nc.sync.dma_start(out_view[:, 0, :], dummy)
                nc.sync.dma_start(ctx_view[:, 0, :], dummy)
                pending.clear()
            # tail: FIFO drain — emission order IS dependency order here
            # (g_fin reads what g_half writes; out-final reads what g_fin
            # writes); the tile scheduler still overlaps across engines
            while pending:
                try:
                    next(pending[0])
                except StopIteration:
                    pending.popleft()
            if debug_dump:
                nc.sync.dma_start(dbg["outmT_d"], outmT_sb)
                nc.sync.dma_start(dbg["ctxmT_d"], ctxmT_sb)
                nc.sync.dma_start(dbg["gsb0_d"], G_sb[0])
                nc.sync.dma_start(dbg["gsb1_d"], G_sb[1])
                nc.sync.dma_start(dbg["qkT_d"], qkT_sb)
                nc.sync.dma_start(dbg["v_d"], v_sb.rearrange("p a b -> p (a b)"))

    nc.compile()
    return nc


_NC_CACHE = {}


def _get_nc():
    if "nc" not in _NC_CACHE:
        _NC_CACHE["nc"] = build_bass()
    return _NC_CACHE["nc"]


def make_in_maps(x, context, W_qk, W_cqk, W_v, W_cv):
    f16 = np.float16
    xT = np.ascontiguousarray(np.asarray(x, np.float32)[0].T).astype(f16)
    cT = np.ascontiguousarray(np.asarray(context, np.float32)[0].T).astype(f16)
    in_maps = []
    for c in range(N_CORES):
        cs = _ts(c, FPC)
        in_maps.append({
            "xT": xT,
            "cT": cT,
            "wqk": np.ascontiguousarray(np.asarray(W_qk)[:, cs]).astype(f16),
            "wv": np.ascontiguousarray(np.asarray(W_v)[:, cs]).astype(f16),
            "wcqk": np.ascontiguousarray(np.asarray(W_cqk)[:, cs]).astype(f16),
            "wcv": np.ascontiguousarray(np.asarray(W_cv)[:, cs]).astype(f16),
        })
    return in_maps


def add_weight_slices(in_maps, W_out, W_cout):
    f16 = np.float16
    for c in range(N_CORES):
        rs = _ts(c, FPC)
        in_maps[c]["wout"] = np.ascontiguousarray(np.asarray(W_out)[rs, :]).astype(f16)
        in_maps[c]["wcout"] = np.ascontiguousarray(np.asarray(W_cout)[rs, :]).astype(f16)
    return in_maps


def kernel(x, context, W_qk, W_cqk, W_v, W_cv, W_out, b_out, W_cout, b_cout):
    from concourse.bass_utils import run_bass_kernel_spmd

    nc = _get_nc()
    in_maps = make_in_maps(x, context, W_qk, W_cqk, W_v, W_cv)
    add_weight_slices(in_maps, W_out, W_cout)

    res = run_bass_kernel_spmd(nc, in_maps, core_ids=list(range(N_CORES)))

    out = np.zeros((SEQ, DIM), np.float32)
    ctx_out = np.zeros((SEQ, DIM), np.float32)
    for r in res.results:
        out += r["out_p"].astype(np.float32)
        ctx_out += r["ctx_p"].astype(np.float32)
    out += np.asarray(b_out, np.float32)
    ctx_out += np.asarray(b_cout, np.float32)
    return (out[None], ctx_out[None])



# revision 38
# speedup vs baseline: 1.3939x; 1.3939x over previous
# Bidirectional cross-attention Trainium2 kernel (Bass/Tile), 8-core head-parallel.
#
# Sharding: 16 heads / 8 cores = 2 heads per core (tensor parallel on h); each
# core computes its heads' projections, similarity, both softmax directions and
# its row-parallel partial of the final projections; host sums partials + bias.
#
# Design:
#  - everything 2-byte fp16 (fp8 fails the 2e-2 gate: elementwise quant noise
#    is preserved through random-sign dot products, measured 5.5e-2)
#  - exp computed ONCE per head (E stored fp16), [128,1024] psum tiles
#  - E^T via DMA xbar transposes emitted as two back-to-back bursts per
#    phase (mid-phase + phase end): bursts keep the fast xbar regime while
#    letting the next phase's g_half consumer start half a phase earlier
#    (measured ~-30us vs one burst per phase)
#  - softmax sums ride along as a ones-column in the V operands
#  - normalization: DVE reciprocal (f32r) + K=1 PE ones-broadcast + DVE mul
#  - half-major phase order (h0,j0)(h1,j0)(h0,j1)(h1,j1): each j-half's ctx
#    rows finish two phases in, so ctx-final projections stream during the
#    phases instead of tailing
#  - G accumulated per j-half into a persistent f32 SBUF accumulator (PSUM
#    chunks + DVE add): only the last head's last half + finals remain after
#    the phase loop
#  - x on the SP HWDGE queue / context on the ACT queue; qk+cqk projections
#    run as the first pass so the first sim phase starts as early as possible
#  - software-pipelined emission: H/G/norm/final work is sprinkled between
#    sim+exp iterations; tail drains FIFO (emission order = dependency order)
#  - phase-resident PSUM->SBUF copies (hT, ctx-final) run on DVE, not ACT:
#    an ACT copy inside the exp stream stalls the pacing engine (measured
#    ~-10us each for hT and ctx-final)

import os
import sys

for _p in ("/opt/trn_rl_repo", "/root/.axon_site/_ro/trn_rl_repo"):
    if os.path.isdir(_p) and _p not in sys.path:
        sys.path.insert(0, _p)

import numpy as np

SEQ_MODE = os.environ.get("KSEQ", "0") == "1"
SPRINKLE_N = int(os.environ.get("KSPR", "8"))
TSPLIT = int(os.environ.get("KTSPLIT", "1"))
WARM = os.environ.get("KWARM", "0") == "1"
HT_DVE = os.environ.get("KHTDVE", "1") == "1"
CF_DVE = os.environ.get("KCFDVE", "1") == "1"
ODMA_GP = os.environ.get("KODMA", "0") == "1"
TQ_ACT = os.environ.get("KTQ", "0") == "1"

HEADS = 16
DIM_HEAD = 64
DIM = 1024
SEQ = 2048
N_CORES = 8
HPC = HEADS // N_CORES          # heads per core = 2
FPC = HPC * DIM_HEAD            # feature cols per core = 128
SCALE = DIM_HEAD ** -0.5


def _ts(i, size):
    return slice(i * size, (i + 1) * size)


def build_bass(seq=SEQ, dim=DIM, fpc=FPC, hpc=HPC, num_devices=N_CORES, stage='full',
               reps=1, debug_dump=False):
    import concourse.bacc as bacc
    import concourse.tile as tile
    import concourse.mybir as mybir
    from contextlib import ExitStack

    f32 = mybir.dt.float32
    f16 = mybir.dt.float16
    Exp = mybir.ActivationFunctionType.Exp

    P = 128
    KT = dim // P              # contraction tiles over DIM (8)
    NT = seq // P              # 128-blocks along sequence (16)
    NCH = seq // 512           # 512-chunks along sequence (4)
    J2 = min(1024, seq)        # exp-tile width
    N2CH = seq // J2           # exp-tile chunks along sequence
    HPT = J2 // 512            # 512-halves per exp tile
    ITPC = NT // NCH           # i-tiles per 512-chunk (4)
    OCH = dim // 512           # 512-chunks of output dim (2)
    dh = DIM_HEAD
    vw = dh + 1

    nc = bacc.Bacc("TRN2", target_bir_lowering=False, debug=False,
                   num_devices=num_devices)

    xT = nc.dram_tensor("xT", (dim, seq), f16, kind="ExternalInput").ap()
    cT = nc.dram_tensor("cT", (dim, seq), f16, kind="ExternalInput").ap()
    wqk = nc.dram_tensor("wqk", (dim, fpc), f16, kind="ExternalInput").ap()
    wv = nc.dram_tensor("wv", (dim, fpc), f16, kind="ExternalInput").ap()
    wcqk = nc.dram_tensor("wcqk", (dim, fpc), f16, kind="ExternalInput").ap()
    wcv = nc.dram_tensor("wcv", (dim, fpc), f16, kind="ExternalInput").ap()
    wout = nc.dram_tensor("wout", (fpc, dim), f16, kind="ExternalInput").ap()
    wcout = nc.dram_tensor("wcout", (fpc, dim), f16, kind="ExternalInput").ap()
    out_p = nc.dram_tensor("out_p", (seq, dim), f16, kind="ExternalOutput").ap()
    ctx_p = nc.dram_tensor("ctx_p", (seq, dim), f16, kind="ExternalOutput").ap()
    dbg = {}
    if debug_dump:
        dbg["outmT_d"] = nc.dram_tensor("outmT_d", (P, seq), f16, kind="ExternalOutput").ap()
        dbg["ctxmT_d"] = nc.dram_tensor("ctxmT_d", (P, seq), f16, kind="ExternalOutput").ap()
        dbg["gsb0_d"] = nc.dram_tensor("gsb0_d", (vw, seq), f32, kind="ExternalOutput").ap()
        dbg["gsb1_d"] = nc.dram_tensor("gsb1_d", (vw, seq), f32, kind="ExternalOutput").ap()
        dbg["qkT_d"] = nc.dram_tensor("qkT_d", (P, seq), f16, kind="ExternalOutput").ap()
        dbg["v_d"] = nc.dram_tensor("v_d", (P, NT * hpc * vw), f16, kind="ExternalOutput").ap()

    with tile.TileContext(nc) as tc:
      for _rep in range(reps):
        with ExitStack() as ctx:
            sfx = f"_r{_rep}"
            persist = ctx.enter_context(tc.tile_pool(name="persist" + sfx, bufs=1))
            fin_pool = ctx.enter_context(tc.tile_pool(name="finpool" + sfx, bufs=4))

            qkT_sb = persist.tile([P, seq], f16, tag="qkT")
            cqkT_sb = persist.tile([P, seq], f16, tag="cqkT")
            v_sb = persist.tile([P, NT, hpc * vw], f16, tag="v")
            cv_sb = persist.tile([P, NT, hpc * vw], f16, tag="cv")
            wout_sb = persist.tile([P, dim], f16, tag="wout")
            wcout_sb = persist.tile([P, dim], f16, tag="wcout")
            outmT_sb = persist.tile([P, seq], f16, tag="outmT")
            ctxmT_sb = persist.tile([P, seq], f16, tag="ctxmT")
            f32r = mybir.dt.float32r
            ones_f = persist.tile([1, dh], f32, tag="onesf", name="ones_f" + sfx)
            nc.vector.memset(ones_f, 1.0)
            ones_r = persist.tile([1, dh], f32r, tag="ones", name="ones_r" + sfx)
            with nc.allow_low_precision(reason="ones constant, exact in f32r"):
                nc.vector.tensor_copy(ones_r, ones_f)

            nc.sync.dma_start(wout_sb, wout)
            nc.sync.dma_start(wcout_sb, wcout)

            # ---- load x/context + weights, compute projections, then release
            with tc.tile_pool(name="xcpool" + sfx, bufs=1) as xc_pool, \
                 tc.tile_pool(name="psproj" + sfx, bufs=8, space="PSUM") as ps_proj:
                w_sbs = {}
                for name, ap_, eng in (("wqk", wqk, nc.sync), ("wcqk", wcqk, nc.scalar),
                                       ("wv", wv, nc.sync), ("wcv", wcv, nc.scalar)):
                    t = xc_pool.tile([P, KT, fpc], f16, tag=name)
                    eng.dma_start(t, ap_.rearrange("(kt p) f -> p kt f", p=P))
                    w_sbs[name] = t
                xT_sb = xc_pool.tile([P, KT, seq], f16, tag="xT")
                cT_sb = xc_pool.tile([P, KT, seq], f16, tag="cT")
                xT_v = xT.rearrange("(kt p) i -> p kt i", p=P)
                cT_v = cT.rearrange("(kt p) i -> p kt i", p=P)
                # x chunks on the SP HWDGE queue, context on the ACT queue:
                # two dispatch streams keep both SDMA fan-outs busy
                for kt in range(KT):
                    nc.sync.dma_start(xT_sb[:, kt], xT_v[:, kt])
                    nc.scalar.dma_start(cT_sb[:, kt], cT_v[:, kt])

                from concourse.masks import make_identity
                ident = persist.tile([P, P], f16, tag="ident")
                make_identity(nc, ident)
                if WARM:
                    # keep the PE clocked up during the input load: ~40 dummy
                    # ident transposes (~2us) so the first projection matmuls
                    # start at the full 2.4GHz p-state instead of ramping
                    wps = ps_proj.tile([P, 512], f32, tag="pp",
                                       name="warm" + sfx)
                    wps16 = wps.bitcast(f16)
                    for wi in range(40):
                        nc.tensor.transpose(wps16[:, 0:P], ident, ident)
                for h in range(hpc):
                    nc.vector.memset(v_sb[:, :, h * vw + dh], 1.0)
                    nc.vector.memset(cv_sb[:, :, h * vw + dh], 1.0)
                vT_tmps = {}
                vT_tmps["wv"] = persist.tile([P, seq], f16, tag="vT_wv", name="vT_wv" + sfx)
                vT_tmps["wcv"] = persist.tile([P, seq], f16, tag="vT_wcv", name="vT_wcv" + sfx)
                # projections in two passes: qk+cqk first (they gate the
                # first sim phase), then v+cv; ktile-major so matmuls chase
                # the input DMAs; 8 psum accumulators live per pass
                proj_passes = (((xT_sb, "wqk", qkT_sb), (cT_sb, "wcqk", cqkT_sb)),
                               ((xT_sb, "wv", vT_tmps["wv"]),
                                (cT_sb, "wcv", vT_tmps["wcv"])))
                for cg, projs in enumerate(proj_passes if stage != 'load' else ()):
                    tiles = {}
                    for pi in range(2):
                        for cc in range(NCH):
                            tiles[(pi, cc)] = ps_proj.tile(
                                [P, 512], f32, tag="pp",
                                name=f"pp_{cg}_{pi}_{cc}" + sfx)
                    for kt in range(KT):
                        for pi, (src_sb, wname, dst) in enumerate(projs):
                            for cc in range(NCH):
                                nc.tensor.matmul(
                                    tiles[(pi, cc)], w_sbs[wname][:, kt],
                                    src_sb[:, kt, _ts(cc, 512)],
                                    start=(kt == 0), stop=(kt == KT - 1))
                    for pi, (src_sb, wname, dst) in enumerate(projs):
                        for cc in range(NCH):
                            nc.vector.tensor_copy(dst[:, _ts(cc, 512)],
                                                  tiles[(pi, cc)])
            ps_pool = ctx.enter_context(
                tc.tile_pool(name="pspool" + sfx, bufs=2, space="PSUM"))
            ps_acc = ctx.enter_context(
                tc.tile_pool(name="psacc" + sfx, bufs=2, space="PSUM"))

            # ---- per-head attention (software-pipelined emission) ----
            # Phases = (head, j-half). Each phase emits sim+exp+transpose for
            # 16 i-tiles; H/G accumulation and normalization work from earlier
            # phases is sprinkled between iterations so PE work rides under
            # the ACT-bound exp stream.
            from collections import deque

            e_pool = ctx.enter_context(tc.tile_pool(name="epool" + sfx, bufs=2))
            et_pool = ctx.enter_context(tc.tile_pool(name="etpool" + sfx, bufs=2))
            hg_pool = ctx.enter_context(tc.tile_pool(name="hgpool" + sfx, bufs=2))
            norm_pool = ctx.enter_context(tc.tile_pool(name="normpool" + sfx, bufs=2))
            JPH = NT // N2CH           # j-tiles per half (8)

            # per-head f32 G accumulators (value rows + sums row), built up
            # half-by-half so the out-side work streams instead of tailing
            G_sb = [persist.tile([vw, seq], f32, tag=f"gsb{h}",
                                 name=f"G_sb{h}" + sfx)
                    for h in range(hpc)]

            def vcv_transpose_work():
                for wname, dst in (("wv", v_sb), ("wcv", cv_sb)):
                    vT_tmp = vT_tmps[wname]
                    for ibg in range(NT // 4):
                        pst = ps_pool.tile([P, 1024], f32, tag="ps")
                        pst16 = pst.bitcast(f16)
                        for k in range(4):
                            nc.tensor.transpose(pst16[:, _ts(k, P)],
                                                vT_tmp[:, _ts(ibg * 4 + k, P)],
                                                ident)
                            yield
                        pstv = pst16[:, :4 * P].rearrange("p (k f) -> p k f", k=4)
                        for h in range(hpc):
                            nc.vector.tensor_copy(
                                dst[:, ibg * 4:(ibg + 1) * 4, h * vw:h * vw + dh],
                                pstv[:, :, h * dh:(h + 1) * dh])
                        yield

            def h_work(h, half, E_half):
                """Accumulate H^T chunks of this (head, j-half) + ctx norm."""
                hs = slice(h * dh, (h + 1) * dh)
                va = slice(h * vw, h * vw + vw)
                hT = hg_pool.tile([vw, J2], f16, tag="ht")
                psH = ps_acc.tile([vw, J2], f32, tag="acc")
                for jcc in range(HPT):
                    jsl_l = _ts(jcc, 512)
                    for it in range(NT):
                        nc.tensor.matmul(psH[:, jsl_l], v_sb[:, it, va],
                                         E_half[:, it, jsl_l],
                                         start=(it == 0), stop=(it == NT - 1))
                        yield
                    if HT_DVE:
                        nc.vector.tensor_copy(hT[:, jsl_l], psH[:, jsl_l])
                    else:
                        nc.scalar.copy(hT[:, jsl_l], psH[:, jsl_l])
                    rcs_r = norm_pool.tile([1, 512], f32r, tag="rc")
                    with nc.allow_low_precision(reason="softmax sums O(2e3); f32r rounding is ~1e-7 rel"):
                        nc.vector.reciprocal(rcs_r, hT[dh:dh + 1, jsl_l])
                    jsl_g = _ts(half * HPT + jcc, 512)
                    bc = ps_pool.tile([P, 1024], f32, tag="ps")
                    nc.tensor.matmul(bc[:dh, :512], ones_r, rcs_r,
                                     start=True, stop=True)
                    nc.vector.tensor_mul(ctxmT_sb[hs, jsl_g], hT[0:dh, jsl_l],
                                         bc[:dh, :512])
                    yield

            def g_half_work(h, half, eTh):
                """Accumulate this j-half's G contribution into G_sb[h]."""
                va = slice(h * vw, h * vw + vw)
                for ihalf in range(N2CH):
                    psG = ps_acc.tile([vw, J2], f32, tag="acc")
                    for icc in range(HPT):
                        isl_l = _ts(icc, 512)
                        isl_g = _ts(ihalf * HPT + icc, 512)
                        for jl in range(JPH):
                            nc.tensor.matmul(psG[:, isl_l], cv_sb[:, half * JPH + jl, va],
                                             eTh[:, jl, isl_g],
                                             start=(jl == 0), stop=(jl == JPH - 1))
                            yield
                    isl_gw = _ts(ihalf, J2)
                    if half == 0:
                        nc.vector.tensor_copy(G_sb[h][:, isl_gw], psG)
                    else:
                        nc.vector.tensor_add(G_sb[h][:, isl_gw], G_sb[h][:, isl_gw],
                                             psG)
                    yield

            def g_fin_work(h):
                """Normalize G_sb[h] into outmT (out-side norm)."""
                hs = slice(h * dh, (h + 1) * dh)
                for icc in range(NCH):
                    isl = _ts(icc, 512)
                    rrs_r = norm_pool.tile([1, 512], f32r, tag="rr")
                    with nc.allow_low_precision(reason="softmax sums O(2e3); f32r rounding is ~1e-7 rel"):
                        nc.vector.reciprocal(rrs_r, G_sb[h][dh:dh + 1, isl])
                    bc2 = ps_pool.tile([P, 1024], f32, tag="ps")
                    nc.tensor.matmul(bc2[:dh, :512], ones_r, rrs_r,
                                     start=True, stop=True)
                    nc.vector.tensor_mul(outmT_sb[hs, isl], G_sb[h][0:dh, isl],
                                         bc2[:dh, :512])
                    yield

            def final_work(mT, w_sb, odram, ibs, all_dve=False):
                for n, ib in enumerate(ibs):
                    pso = ps_pool.tile([P, 1024], f32, tag="ps")
                    for oc in range(OCH):
                        nc.tensor.matmul(pso[:, _ts(oc, 512)], mT[:, _ts(ib, P)],
                                         w_sb[:, _ts(oc, 512)],
                                         start=True, stop=True)
                        yield
                    osb = fin_pool.tile([P, dim], f16, tag="osb")
                    # phase-resident copies go on DVE (an ACT copy would stall
                    # the exp stream that paces the phases); tail copies
                    # alternate ACT/DVE so neither engine serializes the tail
                    if all_dve or n % 2 == 1:
                        nc.vector.tensor_copy(osb, pso[:, :dim])
                    else:
                        nc.scalar.copy(osb, pso[:, :dim])
                    if ODMA_GP:
                        # output DMAs via the idle SWDGE queue so they never
                        # queue behind the eT transpose bursts on SP-HWDGE
                        nc.gpsimd.dma_start(odram[:, ib, :], osb)
                    else:
                        nc.sync.dma_start(odram[:, ib, :], osb)
                    yield

            out_view = out_p.rearrange("(ib p) o -> p ib o", p=P)
            ctx_view = ctx_p.rearrange("(ib p) o -> p ib o", p=P)

            pending = deque()
            if stage != 'load':
                pending.append(vcv_transpose_work())

            def sprinkle(n):
                done = 0
                while pending and done < n:
                    try:
                        next(pending[0])
                        done += 1
                    except StopIteration:
                        pending.popleft()

            # half-major phase order: both heads of a j-half complete
            # back-to-back, so ctx-final columns stream out per half and the
            # per-half G accumulation keeps the out-side off the tail
            phases = [(h, half) for half in range(N2CH) for h in range(hpc)]
            if stage in ('load', 'proj'):
                phases = []
            do_hg = stage not in ('e0', 'e')
            do_fin = stage == 'full'
            for h, half in phases:
                hs = slice(h * dh, (h + 1) * dh)
                E_half = e_pool.tile([P, NT, J2], f16, tag="e")
                eTh = None
                if stage != 'e0':
                    eTh = et_pool.tile([P, JPH, seq], f16, tag="et")
                for it in range(NT):
                    ps = ps_pool.tile([P, 1024], f32, tag="ps")
                    for hlf in range(HPT):
                        js = _ts(half * HPT + hlf, 512)
                        nc.tensor.matmul(ps[:, _ts(hlf, 512)],
                                         qkT_sb[hs, _ts(it, P)],
                                         cqkT_sb[hs, js],
                                         start=True, stop=True)
                    nc.scalar.activation(E_half[:, it, :], ps[:, :J2],
                                         Exp, scale=SCALE)
                    if stage != 'e0' and TQ_ACT:
                        # dispatch from the ACT stream: program order after the
                        # exp that produced this tile, so no cross-engine wait
                        nc.scalar.dma_start_transpose(eTh[:, :, _ts(it, P)],
                                                      E_half[:, it, :])
                    if stage != 'e0' and not TQ_ACT and TSPLIT == 1 and it == NT // 2 - 1:
                        for it2 in range(NT // 2):
                            nc.sync.dma_start_transpose(eTh[:, :, _ts(it2, P)],
                                                        E_half[:, it2, :])
                    if stage != 'e0' and not TQ_ACT and TSPLIT == 2 and it % 4 == 3 and it < NT - 1:
                        for it2 in range(it - 3, it + 1):
                            nc.sync.dma_start_transpose(eTh[:, :, _ts(it2, P)],
                                                        E_half[:, it2, :])
                    sprinkle(SPRINKLE_N)
                if stage != 'e0' and not TQ_ACT:
                    # DMA xbar transposes in two back-to-back bursts (the
                    # fast regime) emitted mid-phase and at phase end, so the
                    # consumer (next phase's g_half) can start on the first
                    # i-half earlier
                    t0 = {0: 0, 1: NT // 2, 2: NT - 4}[TSPLIT]
                    for it in range(t0, NT):
                        nc.sync.dma_start_transpose(eTh[:, :, _ts(it, P)],
                                                    E_half[:, it, :])
                if do_hg:
                    pending.append(h_work(h, half, E_half))
                    pending.append(g_half_work(h, half, eTh))
                    if half == N2CH - 1:
                        pending.append(g_fin_work(h))
                if do_fin and h == hpc - 1:
                    # both heads' H for this j-half are now queued ahead in
                    # FIFO order; this half's ctx-final blocks follow them
                    jb0 = half * (NT // N2CH)
                    pending.append(final_work(ctxmT_sb, wcout_sb, ctx_view,
                                              range(jb0, jb0 + NT // N2CH),
                                              all_dve=CF_DVE and half < N2CH - 1))
                # sequential mode: drain phase work here (coarse-grained sync)
                if SEQ_MODE:
                    while pending:
                        sprinkle(1 << 30)

            if do_fin:
                pending.append(final_work(outmT_sb, wout_sb, out_view, range(NT)))
            else:
                while pending:
                    try:
                        next(pending[0])
                    except StopIteration:
                        pending.popleft()
                dummy = fin_pool.tile([P, dim], f16, tag="osb", name="dummy" + sfx)
                nc.vector.memset(outmT_sb, 0.0)
                nc.vector.memset(ctxmT_sb, 0.0)
                for h in range(hpc):
                    nc.vector.memset(G_sb[h], 0.0)
                nc.vector.memset(dummy, 0.0)
                nc.sync.dma_start(out_view[:, 0, :], dummy)
                nc.sync.dma_start(ctx_view[:, 0, :], dummy)
                pending.clear()
            # tail: FIFO drain — emission order IS dependency order here
            # (g_fin reads what g_half writes; out-final reads what g_fin
            # writes); the tile scheduler still overlaps across engines
            while pending:
                try:
                    next(pending[0])
                except StopIteration:
                    pending.popleft()
            if debug_dump:
                nc.sync.dma_start(dbg["outmT_d"], outmT_sb)
                nc.sync.dma_start(dbg["ctxmT_d"], ctxmT_sb)
                nc.sync.dma_start(dbg["gsb0_d"], G_sb[0])
                nc.sync.dma_start(dbg["gsb1_d"], G_sb[1])
                nc.sync.dma_start(dbg["qkT_d"], qkT_sb)
                nc.sync.dma_start(dbg["v_d"], v_sb.rearrange("p a b -> p (a b)"))

    nc.compile()
    return nc


_NC_CACHE = {}


def _get_nc():
    if "nc" not in _NC_CACHE:
        _NC_CACHE["nc"] = build_bass()
    return _NC_CACHE["nc"]


def make_in_maps(x, context, W_qk, W_cqk, W_v, W_cv):
    f16 = np.float16
    xT = np.ascontiguousarray(np.asarray(x, np.float32)[0].T).astype(f16)
    cT = np.ascontiguousarray(np.asarray(context, np.float32)[0].T).astype(f16)
    in_maps = []
    for c in range(N_CORES):
        cs = _ts(c, FPC)
        in_maps.append({
            "xT": xT,
            "cT": cT,
            "wqk": np.ascontiguousarray(np.asarray(W_qk)[:, cs]).astype(f16),
            "wv": np.ascontiguousarray(np.asarray(W_v)[:, cs]).astype(f16),
            "wcqk": np.ascontiguousarray(np.asarray(W_cqk)[:, cs]).astype(f16),
            "wcv": np.ascontiguousarray(np.asarray(W_cv)[:, cs]).astype(f16),
        })
    return in_maps


def add_weight_slices(in_maps, W_out, W_cout):
    f16 = np.float16
    for c in range(N_CORES):
        rs = _ts(c, FPC)
        in_maps[c]["wout"] = np.ascontiguousarray(np.asarray(W_out)[rs, :]).astype(f16)
        in_maps[c]["wcout"] = np.ascontiguousarray(np.asarray(W_cout)[rs, :]).astype(f16)
    return in_maps


def kernel(x, context, W_qk, W_cqk, W_v, W_cv, W_out, b_out, W_cout, b_cout):
    from concourse.bass_utils import run_bass_kernel_spmd

    nc = _get_nc()
    in_maps = make_in_maps(x, context, W_qk, W_cqk, W_v, W_cv)
    add_weight_slices(in_maps, W_out, W_cout)

    res = run_bass_kernel_spmd(nc, in_maps, core_ids=list(range(N_CORES)))

    out = np.zeros((SEQ, DIM), np.float32)
    ctx_out = np.zeros((SEQ, DIM), np.float32)
    for r in res.results:
        out += r["out_p"].astype(np.float32)
        ctx_out += r["ctx_p"].astype(np.float32)
    out += np.asarray(b_out, np.float32)
    ctx_out += np.asarray(b_cout, np.float32)
    return (out[None], ctx_out[None])

